# revision 28
# baseline (speedup 1.0000x reference)
"""ChronosMOE FeedForward on 8 Trainium2 NeuronCores.

Strategy (expert-parallel, host dispatch/combine — v15):
  - The host computes the router (f32 logits -> top-2 + normalized softmax
    weights) and gathers each expert's tokens; core e receives exactly its
    expert's routed activations (bf16, partition-major) plus its expert
    weights (re-blocked for contiguous DMA, bf16).  The weighted
    scatter-add of expert outputs back into token order (the "combine")
    happens on the host during unsharding, so no collective runs on device
    and the 8 cores execute fully independently.
  - Each core runs two expert sweeps (ragged first sweep sized to the
    actual routing at compile time, then a full 256-token sweep) and one
    shared-expert sweep (its 256 resident tokens).  A sweep is the fused
    g/u SwiGLU + down-projection pipeline in [feature, token] layout:
    persistent PSUM accumulators hold the down-proj output across all 11
    I-tiles; the down-proj lags the g/u stage by one I-tile so the
    silu+mult latency stays off the PE critical path.
  - Expert weights stream on the sync DMA queue with a 2-tile prefetch
    margin; activations and outputs ride the scalar DMA queue so the two
    streams never serialize.  Shared-expert weights stream during the
    second expert sweep.  The first sweep's activations arrive in two
    pieces so the very first matmul starts one transfer earlier; from then
    on the PE must run gap-free or the HAM clock-gate keeps it at 1.2 GHz.
  - Outputs are written back per 128-row chunk as soon as that chunk's
    PSUM chain terminates, so the tail after the last matmul is one
    quarter-width copy + DMA deep.
"""
import numpy as np
import ml_dtypes

import concourse.bass as bass
import concourse.mybir as mybir
import concourse.tile as tile
from concourse import bacc
from concourse.bass_utils import run_bass_kernel_spmd

F32 = mybir.dt.float32
BF16 = mybir.dt.bfloat16
AF = mybir.ActivationFunctionType
OP = mybir.AluOpType

H = 1024          # hidden
E = 8             # experts
I = 1408          # moe intermediate
B, S = 2, 1024
T = B * S         # 2048 tokens
NCORES = 8
HC = H // 128     # 8 H-chunks
IC = I // 128     # 11 I-tiles
PKB = 256         # second expert sweep width (always full)
SST = 256         # shared-expert tokens per core

_CACHE = {}


def _build(pka):
    """pka: first (ragged) expert sweep width, multiple of 4, <= 384."""
    assert pka % 4 == 0 and 0 < pka <= 384
    nc = bacc.Bacc("TRN2", target_bir_lowering=False, debug=False,
                   num_devices=NCORES)

    # activations arrive partition-major ([128, HC, W]) so each sweep's
    # whole gather is one fully-contiguous DMA
    xa_d = nc.dram_tensor("xaT", [128, HC, pka], BF16, kind="ExternalInput")
    xb_d = nc.dram_tensor("xbT", [128, HC, PKB], BF16, kind="ExternalInput")
    xs_d = nc.dram_tensor("xsT", [128, HC, SST], BF16, kind="ExternalInput")
    # up-projection weights, host re-blocked to [IC, 128, H] so each I-tile's
    # stationary [128, hc, 128] group is one contiguous 256 KB DMA
    wgB_d = nc.dram_tensor("wgB", [IC, 128, H], BF16, kind="ExternalInput")
    wuB_d = nc.dram_tensor("wuB", [IC, 128, H], BF16, kind="ExternalInput")
    wgsB_d = nc.dram_tensor("wgsB", [IC, 128, H], BF16, kind="ExternalInput")
    wusB_d = nc.dram_tensor("wusB", [IC, 128, H], BF16, kind="ExternalInput")
    wd_d = nc.dram_tensor("wd", [I, H], BF16, kind="ExternalInput")
    wds_d = nc.dram_tensor("wds", [I, H], BF16, kind="ExternalInput")
    oa_d = nc.dram_tensor("oa", [pka, H], BF16, kind="ExternalOutput")
    ob_d = nc.dram_tensor("ob", [PKB, H], BF16, kind="ExternalOutput")
    os_d = nc.dram_tensor("os", [SST, H], BF16, kind="ExternalOutput")

    pkmax = max(pka, PKB, SST)
    with tile.TileContext(nc) as tc:
        with (
            tc.tile_pool(name="wres", bufs=1) as wres,
            tc.tile_pool(name="act", bufs=1) as act,
            tc.tile_pool(name="small", bufs=2) as small,
            tc.tile_pool(name="htmp", bufs=3) as htmp,
            tc.tile_pool(name="osb", bufs=3) as osb,
            tc.tile_pool(name="psA", bufs=1, space="PSUM") as psA,
            tc.tile_pool(name="psB", bufs=1, space="PSUM") as psB,
        ):
            wg_sb = wres.tile([128, IC, H], BF16, tag="wg")
            wu_sb = wres.tile([128, IC, H], BF16, tag="wu")
            wd_sb = wres.tile([128, IC, H], BF16, tag="wd")
            wgs_sb = wres.tile([128, IC, H], BF16, tag="wgs")
            wus_sb = wres.tile([128, IC, H], BF16, tag="wus")
            wds_sb = wres.tile([128, IC, H], BF16, tag="wds")

            xa_sb = act.tile([128, HC, pka], BF16, tag="xa")
            xb_sb = act.tile([128, HC, PKB], BF16, tag="xb")
            xs_sb = act.tile([128, HC, SST], BF16, tag="xs")

            # startup: weight tiles 0-2 on the sync queue, first-sweep
            # activations in two pieces on the scalar queue
            nc.sync.dma_start(wg_sb[:, 0, :], wgB_d[0])
            nc.scalar.dma_start(xa_sb[:, 0:4, :], xa_d[:, 0:4, :])
            nc.sync.dma_start(wu_sb[:, 0, :], wuB_d[0])
            nc.scalar.dma_start(xa_sb[:, 4:8, :], xa_d[:, 4:8, :])
            nc.sync.dma_start(wd_sb[:, 0, :], wd_d[0:128, :])
            nc.sync.dma_start(wg_sb[:, 1, :], wgB_d[1])
            nc.sync.dma_start(wu_sb[:, 1, :], wuB_d[1])
            nc.sync.dma_start(wd_sb[:, 1, :], wd_d[128:256, :])
            nc.sync.dma_start(wg_sb[:, 2, :], wgB_d[2])
            nc.sync.dma_start(wu_sb[:, 2, :], wuB_d[2])
            nc.sync.dma_start(wd_sb[:, 2, :], wd_d[256:384, :])

            def sweep(x_sb, pk, wg_t, wu_t, wd_t, out_d, mode, name):
                """g/u + fused down-proj over pk tokens (down-proj lagged one
                I-tile).  mode 'first': expert-weight prefetch (3 tiles
                ahead) + staged activation loads; 'second': stream
                shared-expert weights; 'shared': everything resident."""
                nch = (pk + 127) // 128
                obt = [psB.tile([128, 512], F32, tag=f"o{j}",
                                name=f"ob_{name}_{j}")
                       for j in range(2 * nch)]
                h_tiles = [None] * IC

                def down_proj(it, final):
                    for m in range(nch):
                        r = min(128, pk - m * 128)
                        for hn in range(2):
                            nc.tensor.matmul(
                                obt[m * 2 + hn][0:r, :],
                                h_tiles[it][:, m * 128:m * 128 + r],
                                wd_t[:, it, hn * 512:(hn + 1) * 512],
                                start=(it == 0), stop=(it == IC - 1))
                        if final:
                            # vector carries 3 quarters, scalar the last, so
                            # the post-last-matmul chain is one quarter-copy
                            # + DMA deep; each half rides its own DMA queue
                            o_sb = osb.tile([128, H], BF16, tag="o_sb",
                                            name=f"osb_{name}_{m}")
                            nc.vector.tensor_copy(o_sb[0:r, 0:512],
                                                  obt[m * 2][0:r, :])
                            nc.sync.dma_start(
                                out_d[m * 128:m * 128 + r, 0:512],
                                o_sb[0:r, 0:512])
                            nc.vector.tensor_copy(
                                o_sb[0:r, 512:768],
                                obt[m * 2 + 1][0:r, 0:256])
                            nc.scalar.copy(o_sb[0:r, 768:1024],
                                           obt[m * 2 + 1][0:r, 256:512])
                            nc.scalar.dma_start(
                                out_d[m * 128:m * 128 + r, 512:1024],
                                o_sb[0:r, 512:1024])

                for it in range(IC):
                    if mode == "first":
                        # expert weights stay two tiles ahead (tiles it+1,
                        # it+2 already in flight when iteration it starts)
                        if it + 3 <= IC - 1:
                            nc.sync.dma_start(wg_sb[:, it + 3, :],
                                              wgB_d[it + 3])
                            nc.sync.dma_start(wu_sb[:, it + 3, :],
                                              wuB_d[it + 3])
                            nc.sync.dma_start(
                                wd_sb[:, it + 3, :],
                                wd_d[(it + 3) * 128:(it + 4) * 128, :])
                        if it == 2:
                            nc.scalar.dma_start(xb_sb[:], xb_d[:])
                        if it == 4:
                            nc.scalar.dma_start(xs_sb[:], xs_d[:])
                    elif mode == "second":
                        # shared-expert weights: consumed only in the next
                        # phase, so these never gate this sweep's PE
                        nc.sync.dma_start(wgs_sb[:, it, :], wgsB_d[it])
                        nc.sync.dma_start(wus_sb[:, it, :], wusB_d[it])
                        nc.sync.dma_start(wds_sb[:, it, :],
                                          wds_d[it * 128:(it + 1) * 128, :])
                    # pool tiles are allocated at the max sweep width so the
                    # per-tag storage fits every sweep, then sliced to pk
                    g_ps = psA.tile([128, pkmax], F32, tag="g_ps",
                                    name=f"g_{name}_{it}")
                    for hc in range(HC):
                        nc.tensor.matmul(g_ps[:, 0:pk],
                                         wg_t[:, it, hc * 128:(hc + 1) * 128],
                                         x_sb[:, hc, :],
                                         start=(hc == 0), stop=(hc == HC - 1))
                    u_ps = psA.tile([128, pkmax], F32, tag="u_ps",
                                    name=f"u_{name}_{it}")
                    for hc in range(HC):
                        nc.tensor.matmul(u_ps[:, 0:pk],
                                         wu_t[:, it, hc * 128:(hc + 1) * 128],
                                         x_sb[:, hc, :],
                                         start=(hc == 0), stop=(hc == HC - 1))
                    sg = small.tile([128, pkmax], BF16, tag="sg",
                                    name=f"sg_{name}_{it}")
                    nc.scalar.activation(sg[:, 0:pk], g_ps[:, 0:pk], AF.Silu)
                    h0 = htmp.tile([128, pkmax], BF16, tag="h0",
                                   name=f"h_{name}_{it}")
                    nc.vector.tensor_tensor(h0[:, 0:pk], sg[:, 0:pk],
                                            u_ps[:, 0:pk], OP.mult)
                    h_tiles[it] = h0
                    if it > 0:
                        down_proj(it - 1, final=False)
                down_proj(IC - 1, final=True)

            sweep(xa_sb, pka, wg_sb, wu_sb, wd_sb, oa_d, "first", "a")
            sweep(xb_sb, PKB, wg_sb, wu_sb, wd_sb, ob_d, "second", "b")
            sweep(xs_sb, SST, wgs_sb, wus_sb, wds_sb, os_d, "shared", "s")

    nc.compile()
    return nc


def _get_nc(pka):
    key = ("nc", pka)
    if key not in _CACHE:
        _CACHE[key] = _build(pka)
    return _CACHE[key]


def _reblock(w):
    # [H, I] -> [IC, 128, H]: I-tile it's stationary group as one contiguous
    # block: out[it][q, hc*128 + p] = w[hc*128 + q, it*128 + p]
    # (partition q = H index within chunk = contraction dim)
    return np.ascontiguousarray(
        w.reshape(HC, 128, IC, 128).transpose(2, 1, 0, 3).reshape(IC, 128, H)
    ).astype(ml_dtypes.bfloat16)


def _route(x, w_router):
    """Host router: top-2 ids + normalized softmax combine weights."""
    xf = x.reshape(T, H)
    logits = xf @ w_router.T                      # [T, E]
    order = np.argsort(-logits, axis=1, kind="stable")[:, :2]   # top-2 ids
    lg = logits - logits.max(axis=1, keepdims=True)
    sc = np.exp(lg)
    sc /= sc.sum(axis=1, keepdims=True)
    tw = np.take_along_axis(sc, order, axis=1)    # [T, 2]
    tw = tw / (tw.sum(axis=1, keepdims=True) + 1e-20)
    ids, cw = [], []
    for e in range(E):
        hit = order == e                          # [T, 2]
        sel = np.where(hit.any(axis=1))[0]
        ids.append(sel)
        cw.append((tw * hit).sum(axis=1)[sel])
    return ids, cw


def _plan(ids):
    """Split each expert's token list into sweep A (ragged, width pka) and
    sweep B (exactly PKB tokens; fewer only if the expert is tiny)."""
    nmax = max(len(i) for i in ids)
    pka = max(64, -(-max(4, nmax - PKB) // 4) * 4)
    assert pka <= 384, f"routing too imbalanced for 2-sweep plan: {nmax}"
    return pka


def _pmajor(xT, cols):
    """[H, n] gather -> partition-major [128, HC, n] device layout."""
    g = xT[:, cols].reshape(HC, 128, -1).transpose(1, 0, 2)
    return np.ascontiguousarray(g)


def make_in_maps(x, w_router, wg, wu, wd, wg_s, wu_s, wd_s):
    xf = np.asarray(x, np.float32).reshape(T, H)
    ids, cw = _route(xf, np.asarray(w_router, np.float32))
    pka = _plan(ids)
    xT = np.ascontiguousarray(xf.T).astype(ml_dtypes.bfloat16)

    wgsB = _reblock(wg_s)
    wusB = _reblock(wu_s)
    wdsC = np.ascontiguousarray(wd_s).astype(ml_dtypes.bfloat16)

    in_maps, plans = [], []
    for e in range(NCORES):
        n = len(ids[e])
        lb = min(PKB, n)
        la = n - lb
        ia = np.zeros(pka, np.int64)
        ia[:la] = ids[e][:la]
        ib = np.zeros(PKB, np.int64)
        ib[:lb] = ids[e][la:]
        m = {
            "xaT": _pmajor(xT, ia),
            "xbT": _pmajor(xT, ib),
            "xsT": _pmajor(xT, np.arange(e * SST, (e + 1) * SST)),
            "wgB": _reblock(wg[e]),
            "wuB": _reblock(wu[e]),
            "wd": np.ascontiguousarray(wd[e]).astype(ml_dtypes.bfloat16),
            "wgsB": wgsB,
            "wusB": wusB,
            "wds": wdsC,
        }
        in_maps.append(m)
        plans.append((la, lb, ids[e], cw[e]))
    return in_maps, plans, pka


def _combine(res, plans):
    y = np.zeros((T, H), np.float32)
    for e in range(NCORES):
        la, lb, idse, cwe = plans[e]
        oa = np.asarray(res.results[e]["oa"]).astype(np.float32)
        ob = np.asarray(res.results[e]["ob"]).astype(np.float32)
        os_ = np.asarray(res.results[e]["os"]).astype(np.float32)
        if la:
            y[idse[:la]] += cwe[:la, None] * oa[:la]
        if lb:
            y[idse[la:]] += cwe[la:, None] * ob[:lb]
        y[e * SST:(e + 1) * SST] += os_
    return y.reshape(B, S, H)


def _run(inputs, trace=False, tmpdir=None):
    args = [np.asarray(inputs[k], dtype=np.float32) for k in
            ("x", "w_router", "wg", "wu", "wd", "wg_s", "wu_s", "wd_s")]
    in_maps, plans, pka = make_in_maps(*args)
    nc = _get_nc(pka)
    kw = {}
    if trace:
        kw = dict(trace=True, tmpdir=tmpdir)
    res = run_bass_kernel_spmd(nc, in_maps, list(range(NCORES)), **kw)
    return _combine(res, plans), res


def kernel(x, w_router, wg, wu, wd, wg_s, wu_s, wd_s):
    y, _ = _run(dict(x=x, w_router=w_router, wg=wg, wu=wu, wd=wd,
                     wg_s=wg_s, wu_s=wu_s, wd_s=wd_s))
    return y


# revision 29
# speedup vs baseline: 1.1512x; 1.1512x over previous
"""ChronosMOE FeedForward on 8 Trainium2 NeuronCores.

Strategy (expert-parallel, host dispatch/combine — v15):
  - The host computes the router (f32 logits -> top-2 + normalized softmax
    weights) and gathers each expert's tokens; core e receives exactly its
    expert's routed activations (bf16, partition-major) plus its expert
    weights (re-blocked for contiguous DMA, bf16).  The weighted
    scatter-add of expert outputs back into token order (the "combine")
    happens on the host during unsharding, so no collective runs on device
    and the 8 cores execute fully independently.
  - Each core runs two expert sweeps (ragged first sweep sized to the
    actual routing at compile time, then a full 256-token sweep) and one
    shared-expert sweep (its 256 resident tokens).  A sweep is the fused
    g/u SwiGLU + down-projection pipeline in [feature, token] layout:
    persistent PSUM accumulators hold the down-proj output across all 11
    I-tiles; the down-proj lags the g/u stage by one I-tile so the
    silu+mult latency stays off the PE critical path.
  - Expert weights stream on the sync DMA queue with a 2-tile prefetch
    margin; activations and outputs ride the scalar DMA queue so the two
    streams never serialize.  Shared-expert weights stream during the
    second expert sweep.  The first sweep's activations arrive in two
    pieces so the very first matmul starts one transfer earlier; from then
    on the PE must run gap-free or the HAM clock-gate keeps it at 1.2 GHz.
  - Outputs are written back per 128-row chunk as soon as that chunk's
    PSUM chain terminates, so the tail after the last matmul is one
    quarter-width copy + DMA deep.
"""
import numpy as np
import ml_dtypes

import concourse.bass as bass
import concourse.mybir as mybir
import concourse.tile as tile
from concourse import bacc
from concourse.bass_utils import run_bass_kernel_spmd

F32 = mybir.dt.float32
BF16 = mybir.dt.bfloat16
AF = mybir.ActivationFunctionType
OP = mybir.AluOpType

H = 1024          # hidden
E = 8             # experts
I = 1408          # moe intermediate
B, S = 2, 1024
T = B * S         # 2048 tokens
NCORES = 8
HC = H // 128     # 8 H-chunks
IC = I // 128     # 11 I-tiles
PKB = 256         # second expert sweep width (always full)
SST = 256         # shared-expert tokens per core

_CACHE = {}


def _build(pka):
    """pka: first (ragged) expert sweep width, multiple of 4, <= 384."""
    assert pka % 4 == 0 and 0 < pka <= 384
    nc = bacc.Bacc("TRN2", target_bir_lowering=False, debug=False,
                   num_devices=NCORES)

    # activations arrive partition-major ([128, HC, W]) so each sweep's
    # whole gather is one fully-contiguous DMA
    xa_d = nc.dram_tensor("xaT", [128, HC, pka], BF16, kind="ExternalInput")
    xb_d = nc.dram_tensor("xbT", [128, HC, PKB], BF16, kind="ExternalInput")
    xs_d = nc.dram_tensor("xsT", [128, HC, SST], BF16, kind="ExternalInput")
    # up-projection weights, host re-blocked to [IC, 128, H] so each I-tile's
    # stationary [128, hc, 128] group is one contiguous 256 KB DMA
    wgB_d = nc.dram_tensor("wgB", [IC, 128, H], BF16, kind="ExternalInput")
    wuB_d = nc.dram_tensor("wuB", [IC, 128, H], BF16, kind="ExternalInput")
    wgsB_d = nc.dram_tensor("wgsB", [IC, 128, H], BF16, kind="ExternalInput")
    wusB_d = nc.dram_tensor("wusB", [IC, 128, H], BF16, kind="ExternalInput")
    wd_d = nc.dram_tensor("wd", [I, H], BF16, kind="ExternalInput")
    wds_d = nc.dram_tensor("wds", [I, H], BF16, kind="ExternalInput")
    oa_d = nc.dram_tensor("oa", [pka, H], BF16, kind="ExternalOutput")
    ob_d = nc.dram_tensor("ob", [PKB, H], BF16, kind="ExternalOutput")
    os_d = nc.dram_tensor("os", [SST, H], BF16, kind="ExternalOutput")

    pkmax = max(pka, PKB, SST)
    with tile.TileContext(nc) as tc:
        with (
            tc.tile_pool(name="wres", bufs=1) as wres,
            tc.tile_pool(name="act", bufs=1) as act,
            tc.tile_pool(name="small", bufs=2) as small,
            tc.tile_pool(name="htmp", bufs=3) as htmp,
            tc.tile_pool(name="osb", bufs=3) as osb,
            tc.tile_pool(name="psA", bufs=1, space="PSUM") as psA,
            tc.tile_pool(name="psB", bufs=1, space="PSUM") as psB,
        ):
            wg_sb = wres.tile([128, IC, H], BF16, tag="wg")
            wu_sb = wres.tile([128, IC, H], BF16, tag="wu")
            wd_sb = wres.tile([128, IC, H], BF16, tag="wd")
            wgs_sb = wres.tile([128, IC, H], BF16, tag="wgs")
            wus_sb = wres.tile([128, IC, H], BF16, tag="wus")
            wds_sb = wres.tile([128, IC, H], BF16, tag="wds")

            xa_sb = act.tile([128, HC, pka], BF16, tag="xa")
            xb_sb = act.tile([128, HC, PKB], BF16, tag="xb")
            xs_sb = act.tile([128, HC, SST], BF16, tag="xs")

            # startup: weight tiles 0-2 on the sync queue, first-sweep
            # activations in two pieces on the scalar queue
            nc.sync.dma_start(wg_sb[:, 0, :], wgB_d[0])
            nc.scalar.dma_start(xa_sb[:, 0:4, :], xa_d[:, 0:4, :])
            nc.sync.dma_start(wu_sb[:, 0, :], wuB_d[0])
            nc.scalar.dma_start(xa_sb[:, 4:8, :], xa_d[:, 4:8, :])
            nc.sync.dma_start(wd_sb[:, 0, :], wd_d[0:128, :])
            nc.sync.dma_start(wg_sb[:, 1, :], wgB_d[1])
            nc.sync.dma_start(wu_sb[:, 1, :], wuB_d[1])
            nc.sync.dma_start(wd_sb[:, 1, :], wd_d[128:256, :])
            nc.sync.dma_start(wg_sb[:, 2, :], wgB_d[2])
            nc.sync.dma_start(wu_sb[:, 2, :], wuB_d[2])
            nc.sync.dma_start(wd_sb[:, 2, :], wd_d[256:384, :])

            def sweep(x_sb, pk, wg_t, wu_t, wd_t, out_d, mode, name):
                """g/u + fused down-proj over pk tokens (down-proj lagged one
                I-tile).  mode 'first': expert-weight prefetch (3 tiles
                ahead) + staged activation loads; 'second': stream
                shared-expert weights; 'shared': everything resident."""
                nch = (pk + 127) // 128
                obt = [psB.tile([128, 512], F32, tag=f"o{j}",
                                name=f"ob_{name}_{j}")
                       for j in range(2 * nch)]
                h_tiles = [None] * IC

                def down_proj(it, final):
                    # hn outer: consecutive matmuls never share a stationary
                    # operand, so every LDWEIGHTS can pull ahead during the
                    # previous matmul's stream
                    for hn in range(2):
                        for m in range(nch):
                            r = min(128, pk - m * 128)
                            nc.tensor.matmul(
                                obt[m * 2 + hn][0:r, :],
                                h_tiles[it][:, m * 128:m * 128 + r],
                                wd_t[:, it, hn * 512:(hn + 1) * 512],
                                start=(it == 0), stop=(it == IC - 1))
                    for m in range(nch):
                        r = min(128, pk - m * 128)
                        if final:
                            # vector carries 3 quarters, scalar the last, so
                            # the post-last-matmul chain is one quarter-copy
                            # + DMA deep; each half rides its own DMA queue
                            o_sb = osb.tile([128, H], BF16, tag="o_sb",
                                            name=f"osb_{name}_{m}")
                            nc.vector.tensor_copy(o_sb[0:r, 0:512],
                                                  obt[m * 2][0:r, :])
                            nc.sync.dma_start(
                                out_d[m * 128:m * 128 + r, 0:512],
                                o_sb[0:r, 0:512])
                            nc.vector.tensor_copy(
                                o_sb[0:r, 512:768],
                                obt[m * 2 + 1][0:r, 0:256])
                            nc.scalar.copy(o_sb[0:r, 768:1024],
                                           obt[m * 2 + 1][0:r, 256:512])
                            nc.scalar.dma_start(
                                out_d[m * 128:m * 128 + r, 512:1024],
                                o_sb[0:r, 512:1024])

                for it in range(IC):
                    if mode == "first":
                        # expert weights stay two tiles ahead (tiles it+1,
                        # it+2 already in flight when iteration it starts)
                        if it + 3 <= IC - 1:
                            nc.sync.dma_start(wg_sb[:, it + 3, :],
                                              wgB_d[it + 3])
                            nc.sync.dma_start(wu_sb[:, it + 3, :],
                                              wuB_d[it + 3])
                            nc.sync.dma_start(
                                wd_sb[:, it + 3, :],
                                wd_d[(it + 3) * 128:(it + 4) * 128, :])
                        if it == 2:
                            nc.scalar.dma_start(xb_sb[:], xb_d[:])
                        if it == 4:
                            nc.scalar.dma_start(xs_sb[:], xs_d[:])
                    elif mode == "second":
                        # shared-expert weights: consumed only in the next
                        # phase, so these never gate this sweep's PE
                        nc.sync.dma_start(wgs_sb[:, it, :], wgsB_d[it])
                        nc.sync.dma_start(wus_sb[:, it, :], wusB_d[it])
                        nc.sync.dma_start(wds_sb[:, it, :],
                                          wds_d[it * 128:(it + 1) * 128, :])
                    # pool tiles are allocated at the max sweep width so the
                    # per-tag storage fits every sweep, then sliced to pk
                    g_ps = psA.tile([128, pkmax], F32, tag="g_ps",
                                    name=f"g_{name}_{it}")
                    for hc in range(HC):
                        nc.tensor.matmul(g_ps[:, 0:pk],
                                         wg_t[:, it, hc * 128:(hc + 1) * 128],
                                         x_sb[:, hc, :],
                                         start=(hc == 0), stop=(hc == HC - 1))
                    u_ps = psA.tile([128, pkmax], F32, tag="u_ps",
                                    name=f"u_{name}_{it}")
                    for hc in range(HC):
                        nc.tensor.matmul(u_ps[:, 0:pk],
                                         wu_t[:, it, hc * 128:(hc + 1) * 128],
                                         x_sb[:, hc, :],
                                         start=(hc == 0), stop=(hc == HC - 1))
                    sg = small.tile([128, pkmax], BF16, tag="sg",
                                    name=f"sg_{name}_{it}")
                    nc.scalar.activation(sg[:, 0:pk], g_ps[:, 0:pk], AF.Silu)
                    h0 = htmp.tile([128, pkmax], BF16, tag="h0",
                                   name=f"h_{name}_{it}")
                    nc.vector.tensor_tensor(h0[:, 0:pk], sg[:, 0:pk],
                                            u_ps[:, 0:pk], OP.mult)
                    h_tiles[it] = h0
                    if it > 0:
                        down_proj(it - 1, final=False)
                down_proj(IC - 1, final=True)

            sweep(xa_sb, pka, wg_sb, wu_sb, wd_sb, oa_d, "first", "a")
            sweep(xb_sb, PKB, wg_sb, wu_sb, wd_sb, ob_d, "second", "b")
            sweep(xs_sb, SST, wgs_sb, wus_sb, wds_sb, os_d, "shared", "s")

    nc.compile()
    return nc


def _get_nc(pka):
    key = ("nc", pka)
    if key not in _CACHE:
        _CACHE[key] = _build(pka)
    return _CACHE[key]


def _reblock(w):
    # [H, I] -> [IC, 128, H]: I-tile it's stationary group as one contiguous
    # block: out[it][q, hc*128 + p] = w[hc*128 + q, it*128 + p]
    # (partition q = H index within chunk = contraction dim)
    return np.ascontiguousarray(
        w.reshape(HC, 128, IC, 128).transpose(2, 1, 0, 3).reshape(IC, 128, H)
    ).astype(ml_dtypes.bfloat16)


def _route(x, w_router):
    """Host router: top-2 ids + normalized softmax combine weights."""
    xf = x.reshape(T, H)
    logits = xf @ w_router.T                      # [T, E]
    order = np.argsort(-logits, axis=1, kind="stable")[:, :2]   # top-2 ids
    lg = logits - logits.max(axis=1, keepdims=True)
    sc = np.exp(lg)
    sc /= sc.sum(axis=1, keepdims=True)
    tw = np.take_along_axis(sc, order, axis=1)    # [T, 2]
    tw = tw / (tw.sum(axis=1, keepdims=True) + 1e-20)
    ids, cw = [], []
    for e in range(E):
        hit = order == e                          # [T, 2]
        sel = np.where(hit.any(axis=1))[0]
        ids.append(sel)
        cw.append((tw * hit).sum(axis=1)[sel])
    return ids, cw


def _plan(ids):
    """Split each expert's token list into sweep A (ragged, width pka) and
    sweep B (exactly PKB tokens; fewer only if the expert is tiny)."""
    nmax = max(len(i) for i in ids)
    pka = max(64, -(-max(4, nmax - PKB) // 4) * 4)
    assert pka <= 384, f"routing too imbalanced for 2-sweep plan: {nmax}"
    return pka


def _pmajor(xT, cols):
    """[H, n] gather -> partition-major [128, HC, n] device layout."""
    g = xT[:, cols].reshape(HC, 128, -1).transpose(1, 0, 2)
    return np.ascontiguousarray(g)


def make_in_maps(x, w_router, wg, wu, wd, wg_s, wu_s, wd_s):
    xf = np.asarray(x, np.float32).reshape(T, H)
    ids, cw = _route(xf, np.asarray(w_router, np.float32))
    pka = _plan(ids)
    xT = np.ascontiguousarray(xf.T).astype(ml_dtypes.bfloat16)

    wgsB = _reblock(wg_s)
    wusB = _reblock(wu_s)
    wdsC = np.ascontiguousarray(wd_s).astype(ml_dtypes.bfloat16)

    in_maps, plans = [], []
    for e in range(NCORES):
        n = len(ids[e])
        lb = min(PKB, n)
        la = n - lb
        ia = np.zeros(pka, np.int64)
        ia[:la] = ids[e][:la]
        ib = np.zeros(PKB, np.int64)
        ib[:lb] = ids[e][la:]
        m = {
            "xaT": _pmajor(xT, ia),
            "xbT": _pmajor(xT, ib),
            "xsT": _pmajor(xT, np.arange(e * SST, (e + 1) * SST)),
            "wgB": _reblock(wg[e]),
            "wuB": _reblock(wu[e]),
            "wd": np.ascontiguousarray(wd[e]).astype(ml_dtypes.bfloat16),
            "wgsB": wgsB,
            "wusB": wusB,
            "wds": wdsC,
        }
        in_maps.append(m)
        plans.append((la, lb, ids[e], cw[e]))
    return in_maps, plans, pka


def _combine(res, plans):
    y = np.zeros((T, H), np.float32)
    for e in range(NCORES):
        la, lb, idse, cwe = plans[e]
        oa = np.asarray(res.results[e]["oa"]).astype(np.float32)
        ob = np.asarray(res.results[e]["ob"]).astype(np.float32)
        os_ = np.asarray(res.results[e]["os"]).astype(np.float32)
        if la:
            y[idse[:la]] += cwe[:la, None] * oa[:la]
        if lb:
            y[idse[la:]] += cwe[la:, None] * ob[:lb]
        y[e * SST:(e + 1) * SST] += os_
    return y.reshape(B, S, H)


def _run(inputs, trace=False, tmpdir=None):
    args = [np.asarray(inputs[k], dtype=np.float32) for k in
            ("x", "w_router", "wg", "wu", "wd", "wg_s", "wu_s", "wd_s")]
    in_maps, plans, pka = make_in_maps(*args)
    nc = _get_nc(pka)
    kw = {}
    if trace:
        kw = dict(trace=True, tmpdir=tmpdir)
    res = run_bass_kernel_spmd(nc, in_maps, list(range(NCORES)), **kw)
    return _combine(res, plans), res


def kernel(x, w_router, wg, wu, wd, wg_s, wu_s, wd_s):
    y, _ = _run(dict(x=x, w_router=w_router, wg=wg, wu=wu, wd=wd,
                     wg_s=wg_s, wu_s=wu_s, wd_s=wd_s))
    return y


# revision 30
# speedup vs baseline: 1.1671x; 1.0138x over previous
"""ChronosMOE FeedForward on 8 Trainium2 NeuronCores.

Strategy (expert-parallel, host dispatch/combine — v15):
  - The host computes the router (f32 logits -> top-2 + normalized softmax
    weights) and gathers each expert's tokens; core e receives exactly its
    expert's routed activations (bf16, partition-major) plus its expert
    weights (re-blocked for contiguous DMA, bf16).  The weighted
    scatter-add of expert outputs back into token order (the "combine")
    happens on the host during unsharding, so no collective runs on device
    and the 8 cores execute fully independently.
  - Each core runs two expert sweeps (ragged first sweep sized to the
    actual routing at compile time, then a full 256-token sweep) and one
    shared-expert sweep (its 256 resident tokens).  A sweep is the fused
    g/u SwiGLU + down-projection pipeline in [feature, token] layout:
    persistent PSUM accumulators hold the down-proj output across all 11
    I-tiles; the down-proj lags the g/u stage by one I-tile so the
    silu+mult latency stays off the PE critical path.
  - Expert weights stream on the sync DMA queue with a 2-tile prefetch
    margin; activations and outputs ride the scalar DMA queue so the two
    streams never serialize.  Shared-expert weights stream during the
    second expert sweep.  The first sweep's activations arrive in two
    pieces so the very first matmul starts one transfer earlier; from then
    on the PE must run gap-free or the HAM clock-gate keeps it at 1.2 GHz.
  - Outputs are written back per 128-row chunk as soon as that chunk's
    PSUM chain terminates, so the tail after the last matmul is one
    quarter-width copy + DMA deep.
"""
import numpy as np
import ml_dtypes

import concourse.bass as bass
import concourse.mybir as mybir
import concourse.tile as tile
from concourse import bacc
from concourse.bass_utils import run_bass_kernel_spmd

F32 = mybir.dt.float32
BF16 = mybir.dt.bfloat16
AF = mybir.ActivationFunctionType
OP = mybir.AluOpType

H = 1024          # hidden
E = 8             # experts
I = 1408          # moe intermediate
B, S = 2, 1024
T = B * S         # 2048 tokens
NCORES = 8
HC = H // 128     # 8 H-chunks
IC = I // 128     # 11 I-tiles
PKB = 256         # second expert sweep width (always full)
SST = 256         # shared-expert tokens per core

_CACHE = {}


def _build(pka):
    """pka: first (ragged) expert sweep width, multiple of 4, <= 384."""
    assert pka % 4 == 0 and 0 < pka <= 384
    nc = bacc.Bacc("TRN2", target_bir_lowering=False, debug=False,
                   num_devices=NCORES)

    # activations arrive partition-major ([128, HC, W]) so each sweep's
    # whole gather is one fully-contiguous DMA
    xa_d = nc.dram_tensor("xaT", [128, HC, pka], BF16, kind="ExternalInput")
    xb_d = nc.dram_tensor("xbT", [128, HC, PKB], BF16, kind="ExternalInput")
    xs_d = nc.dram_tensor("xsT", [128, HC, SST], BF16, kind="ExternalInput")
    # up-projection weights, host re-blocked to [IC, 128, H] so each I-tile's
    # stationary [128, hc, 128] group is one contiguous 256 KB DMA
    wgB_d = nc.dram_tensor("wgB", [IC, 128, H], BF16, kind="ExternalInput")
    wuB_d = nc.dram_tensor("wuB", [IC, 128, H], BF16, kind="ExternalInput")
    wgsB_d = nc.dram_tensor("wgsB", [IC, 128, H], BF16, kind="ExternalInput")
    wusB_d = nc.dram_tensor("wusB", [IC, 128, H], BF16, kind="ExternalInput")
    wd_d = nc.dram_tensor("wd", [I, H], BF16, kind="ExternalInput")
    wds_d = nc.dram_tensor("wds", [I, H], BF16, kind="ExternalInput")
    oa_d = nc.dram_tensor("oa", [pka, H], BF16, kind="ExternalOutput")
    ob_d = nc.dram_tensor("ob", [PKB, H], BF16, kind="ExternalOutput")
    os_d = nc.dram_tensor("os", [SST, H], BF16, kind="ExternalOutput")

    pkmax = max(pka, PKB, SST)
    with tile.TileContext(nc) as tc:
        with (
            tc.tile_pool(name="wres", bufs=1) as wres,
            tc.tile_pool(name="act", bufs=1) as act,
            tc.tile_pool(name="small", bufs=2) as small,
            tc.tile_pool(name="htmp", bufs=3) as htmp,
            tc.tile_pool(name="osb", bufs=3) as osb,
            tc.tile_pool(name="psA", bufs=1, space="PSUM") as psA,
            tc.tile_pool(name="psB", bufs=1, space="PSUM") as psB,
        ):
            wg_sb = wres.tile([128, IC, H], BF16, tag="wg")
            wu_sb = wres.tile([128, IC, H], BF16, tag="wu")
            wd_sb = wres.tile([128, IC, H], BF16, tag="wd")
            wgs_sb = wres.tile([128, IC, H], BF16, tag="wgs")
            wus_sb = wres.tile([128, IC, H], BF16, tag="wus")
            wds_sb = wres.tile([128, IC, H], BF16, tag="wds")

            xa_sb = act.tile([128, HC, pka], BF16, tag="xa")
            xb_sb = act.tile([128, HC, PKB], BF16, tag="xb")
            xs_sb = act.tile([128, HC, SST], BF16, tag="xs")

            # startup: weight tiles 0-2 on the sync queue, first-sweep
            # activations in two pieces on the scalar queue
            nc.sync.dma_start(wg_sb[:, 0, :], wgB_d[0])
            nc.scalar.dma_start(xa_sb[:, 0:4, :], xa_d[:, 0:4, :])
            nc.sync.dma_start(wu_sb[:, 0, :], wuB_d[0])
            nc.scalar.dma_start(xa_sb[:, 4:8, :], xa_d[:, 4:8, :])
            nc.sync.dma_start(wd_sb[:, 0, :], wd_d[0:128, :])
            nc.sync.dma_start(wg_sb[:, 1, :], wgB_d[1])
            nc.sync.dma_start(wu_sb[:, 1, :], wuB_d[1])
            nc.sync.dma_start(wd_sb[:, 1, :], wd_d[128:256, :])
            nc.sync.dma_start(wg_sb[:, 2, :], wgB_d[2])
            nc.sync.dma_start(wu_sb[:, 2, :], wuB_d[2])
            nc.sync.dma_start(wd_sb[:, 2, :], wd_d[256:384, :])

            def sweep(x_sb, pk, wg_t, wu_t, wd_t, out_d, mode, name):
                """g/u + fused down-proj over pk tokens (down-proj lagged one
                I-tile).  mode 'first': expert-weight prefetch (3 tiles
                ahead) + staged activation loads; 'second': stream
                shared-expert weights; 'shared': everything resident."""
                nch = (pk + 127) // 128
                obt = [psB.tile([128, 512], F32, tag=f"o{j}",
                                name=f"ob_{name}_{j}")
                       for j in range(2 * nch)]
                h_tiles = [None] * IC

                def down_proj(it, final):
                    for m in range(nch):
                        r = min(128, pk - m * 128)
                        for hn in range(2):
                            nc.tensor.matmul(
                                obt[m * 2 + hn][0:r, :],
                                h_tiles[it][:, m * 128:m * 128 + r],
                                wd_t[:, it, hn * 512:(hn + 1) * 512],
                                start=(it == 0), stop=(it == IC - 1))
                        if final:
                            # vector carries 3 quarters, scalar the last, so
                            # the post-last-matmul chain is one quarter-copy
                            # + DMA deep; each half rides its own DMA queue
                            o_sb = osb.tile([128, H], BF16, tag="o_sb",
                                            name=f"osb_{name}_{m}")
                            nc.vector.tensor_copy(o_sb[0:r, 0:512],
                                                  obt[m * 2][0:r, :])
                            nc.sync.dma_start(
                                out_d[m * 128:m * 128 + r, 0:512],
                                o_sb[0:r, 0:512])
                            nc.vector.tensor_copy(
                                o_sb[0:r, 512:768],
                                obt[m * 2 + 1][0:r, 0:256])
                            nc.scalar.copy(o_sb[0:r, 768:1024],
                                           obt[m * 2 + 1][0:r, 256:512])
                            nc.scalar.dma_start(
                                out_d[m * 128:m * 128 + r, 512:1024],
                                o_sb[0:r, 512:1024])

                for it in range(IC):
                    if mode == "first":
                        # expert weights stay two tiles ahead (tiles it+1,
                        # it+2 already in flight when iteration it starts)
                        if it + 3 <= IC - 1:
                            nc.sync.dma_start(wg_sb[:, it + 3, :],
                                              wgB_d[it + 3])
                            nc.sync.dma_start(wu_sb[:, it + 3, :],
                                              wuB_d[it + 3])
                            nc.sync.dma_start(
                                wd_sb[:, it + 3, :],
                                wd_d[(it + 3) * 128:(it + 4) * 128, :])
                        if it == 2:
                            nc.scalar.dma_start(xb_sb[:], xb_d[:])
                        if it == 4:
                            nc.scalar.dma_start(xs_sb[:], xs_d[:])
                    elif mode == "second":
                        # shared-expert weights: consumed only in the next
                        # phase, so these never gate this sweep's PE
                        nc.sync.dma_start(wgs_sb[:, it, :], wgsB_d[it])
                        nc.sync.dma_start(wus_sb[:, it, :], wusB_d[it])
                        nc.sync.dma_start(wds_sb[:, it, :],
                                          wds_d[it * 128:(it + 1) * 128, :])
                    # pool tiles are allocated at the max sweep width so the
                    # per-tag storage fits every sweep, then sliced to pk
                    g_ps = psA.tile([128, pkmax], F32, tag="g_ps",
                                    name=f"g_{name}_{it}")
                    for hc in range(HC):
                        nc.tensor.matmul(g_ps[:, 0:pk],
                                         wg_t[:, it, hc * 128:(hc + 1) * 128],
                                         x_sb[:, hc, :],
                                         start=(hc == 0), stop=(hc == HC - 1))
                    u_ps = psA.tile([128, pkmax], F32, tag="u_ps",
                                    name=f"u_{name}_{it}")
                    for hc in range(HC):
                        nc.tensor.matmul(u_ps[:, 0:pk],
                                         wu_t[:, it, hc * 128:(hc + 1) * 128],
                                         x_sb[:, hc, :],
                                         start=(hc == 0), stop=(hc == HC - 1))
                    sg = small.tile([128, pkmax], BF16, tag="sg",
                                    name=f"sg_{name}_{it}")
                    nc.scalar.activation(sg[:, 0:pk], g_ps[:, 0:pk], AF.Silu)
                    h0 = htmp.tile([128, pkmax], BF16, tag="h0",
                                   name=f"h_{name}_{it}")
                    nc.vector.tensor_tensor(h0[:, 0:pk], sg[:, 0:pk],
                                            u_ps[:, 0:pk], OP.mult)
                    h_tiles[it] = h0
                    if it > 0:
                        down_proj(it - 1, final=False)
                down_proj(IC - 1, final=True)

            sweep(xa_sb, pka, wg_sb, wu_sb, wd_sb, oa_d, "first", "a")
            sweep(xb_sb, PKB, wg_sb, wu_sb, wd_sb, ob_d, "second", "b")
            sweep(xs_sb, SST, wgs_sb, wus_sb, wds_sb, os_d, "shared", "s")

    nc.compile()
    return nc


def _get_nc(pka):
    key = ("nc", pka)
    if key not in _CACHE:
        _CACHE[key] = _build(pka)
    return _CACHE[key]


def _reblock(w):
    # [H, I] -> [IC, 128, H]: I-tile it's stationary group as one contiguous
    # block: out[it][q, hc*128 + p] = w[hc*128 + q, it*128 + p]
    # (partition q = H index within chunk = contraction dim)
    return np.ascontiguousarray(
        w.reshape(HC, 128, IC, 128).transpose(2, 1, 0, 3).reshape(IC, 128, H)
    ).astype(ml_dtypes.bfloat16)


def _route(x, w_router):
    """Host router: top-2 ids + normalized softmax combine weights."""
    xf = x.reshape(T, H)
    logits = xf @ w_router.T                      # [T, E]
    order = np.argsort(-logits, axis=1, kind="stable")[:, :2]   # top-2 ids
    lg = logits - logits.max(axis=1, keepdims=True)
    sc = np.exp(lg)
    sc /= sc.sum(axis=1, keepdims=True)
    tw = np.take_along_axis(sc, order, axis=1)    # [T, 2]
    tw = tw / (tw.sum(axis=1, keepdims=True) + 1e-20)
    ids, cw = [], []
    for e in range(E):
        hit = order == e                          # [T, 2]
        sel = np.where(hit.any(axis=1))[0]
        ids.append(sel)
        cw.append((tw * hit).sum(axis=1)[sel])
    return ids, cw


def _plan(ids):
    """Split each expert's token list into sweep A (ragged, width pka) and
    sweep B (exactly PKB tokens; fewer only if the expert is tiny)."""
    nmax = max(len(i) for i in ids)
    pka = max(64, -(-max(4, nmax - PKB) // 4) * 4)
    assert pka <= 384, f"routing too imbalanced for 2-sweep plan: {nmax}"
    return pka


def _pmajor(xT, cols):
    """[H, n] gather -> partition-major [128, HC, n] device layout."""
    g = xT[:, cols].reshape(HC, 128, -1).transpose(1, 0, 2)
    return np.ascontiguousarray(g)


def make_in_maps(x, w_router, wg, wu, wd, wg_s, wu_s, wd_s):
    xf = np.asarray(x, np.float32).reshape(T, H)
    ids, cw = _route(xf, np.asarray(w_router, np.float32))
    pka = _plan(ids)
    xT = np.ascontiguousarray(xf.T).astype(ml_dtypes.bfloat16)

    wgsB = _reblock(wg_s)
    wusB = _reblock(wu_s)
    wdsC = np.ascontiguousarray(wd_s).astype(ml_dtypes.bfloat16)

    in_maps, plans = [], []
    for e in range(NCORES):
        n = len(ids[e])
        lb = min(PKB, n)
        la = n - lb
        ia = np.zeros(pka, np.int64)
        ia[:la] = ids[e][:la]
        ib = np.zeros(PKB, np.int64)
        ib[:lb] = ids[e][la:]
        m = {
            "xaT": _pmajor(xT, ia),
            "xbT": _pmajor(xT, ib),
            "xsT": _pmajor(xT, np.arange(e * SST, (e + 1) * SST)),
            "wgB": _reblock(wg[e]),
            "wuB": _reblock(wu[e]),
            "wd": np.ascontiguousarray(wd[e]).astype(ml_dtypes.bfloat16),
            "wgsB": wgsB,
            "wusB": wusB,
            "wds": wdsC,
        }
        in_maps.append(m)
        plans.append((la, lb, ids[e], cw[e]))
    return in_maps, plans, pka


def _combine(res, plans):
    y = np.zeros((T, H), np.float32)
    for e in range(NCORES):
        la, lb, idse, cwe = plans[e]
        oa = np.asarray(res.results[e]["oa"]).astype(np.float32)
        ob = np.asarray(res.results[e]["ob"]).astype(np.float32)
        os_ = np.asarray(res.results[e]["os"]).astype(np.float32)
        if la:
            y[idse[:la]] += cwe[:la, None] * oa[:la]
        if lb:
            y[idse[la:]] += cwe[la:, None] * ob[:lb]
        y[e * SST:(e + 1) * SST] += os_
    return y.reshape(B, S, H)


def _run(inputs, trace=False, tmpdir=None):
    args = [np.asarray(inputs[k], dtype=np.float32) for k in
            ("x", "w_router", "wg", "wu", "wd", "wg_s", "wu_s", "wd_s")]
    in_maps, plans, pka = make_in_maps(*args)
    nc = _get_nc(pka)
    kw = {}
    if trace:
        kw = dict(trace=True, tmpdir=tmpdir)
    res = run_bass_kernel_spmd(nc, in_maps, list(range(NCORES)), **kw)
    return _combine(res, plans), res


def kernel(x, w_router, wg, wu, wd, wg_s, wu_s, wd_s):
    y, _ = _run(dict(x=x, w_router=w_router, wg=wg, wu=wu, wd=wd,
                     wg_s=wg_s, wu_s=wu_s, wd_s=wd_s))
    return y
